# revision 2
# baseline (speedup 1.0000x reference)
"""Causal attention head (B=4, T=4096, D=1024, H=64) on 8 TRN2 NeuronCores.

Sharding: 2 cores per batch element, block-interleaved. Core role r in {0,1}
owns 128-row query blocks {[256u+128r, 256u+128r+128) : u in [0,16)}.

Host permutes x^T's 128-column blocks so each core sees its OWN query blocks
at even positions: permuted block 2u = global block 2u+r (own, "A"), block
2u+1 = global block 2u+1-r (partner, "B"). This makes the program
SPMD-uniform: q-tile u attends permuted key tiles {A(0..u), B(0..u)} where
A(u) carries a lower-triangular causal mask and B(u) an all-zero (r=0) /
all-one (r=1) data mask. Q projects straight out of the permuted x^T's even
blocks, so only ONE 8 MiB x^T load per core (no separate x_q load).

Per-core device program:
  - weights (wkv/wq/masks/ident) DMA'd first on the scalar queue;
    x^T [D,T] bf16 in 8 slices on the sync queue
  - K^T/V^T projection per 512-col slice (lhsT=[Wk|Wv]), lazily interleaved
    into the attention phases; V^T -> V via PE transposes into [128,65]
    tiles with a fused ones column (softmax denominator)
  - Q^T projection from even blocks via 2-level strided APs, paced by slice
    arrival
  - flash-style attention in 6 query-column phases (128/128/256/512/512/512
    wide). Per q-tile u, the A/B key-tile strips share one PSUM tile and a
    single exp (ScalarE, scale=1/8); causal masks multiply on the diagonal
    128 cols; ctx^T accumulated [65, W] in PSUM per phase
  - per-phase epilogue: copy ctx^T to SBUF, DMA out raw [num;den] rows
Host side: permute/cast inputs, gather + divide by denominator row.
"""

import numpy as np
import ml_dtypes

import concourse.tile as tile
import concourse.mybir as mybir
from concourse import bacc
from concourse.bass_utils import run_bass_kernel_spmd

BF16 = ml_dtypes.bfloat16
F32 = np.float32

B, T, D, H = 4, 4096, 1024, 64
TL = 2048          # local query columns per core
N_CORES = 8
NKT = T // 128     # 32 key tiles (permuted order)
NU = TL // 128     # 16 own query tiles
DCH = D // 128     # 8 contraction chunks
DT_BF = mybir.dt.bfloat16
DT_F32 = mybir.dt.float32
EXP = mybir.ActivationFunctionType.Exp
MUL = mybir.AluOpType.mult

# attention phases: absolute local q-col ranges, each width <= 512
PHASES = [(0, 128), (128, 256), (256, 512),
          (512, 1024), (1024, 1536), (1536, 2048)]
# Q emissions: (u0, ntiles) -> requires xt slices floor(u0/2) .. floor((u0+n-1)/2)
QEMITS = [(0, 2), (2, 2), (4, 4), (8, 4), (12, 4)]


def _build():
    nc = bacc.Bacc("TRN2", target_bir_lowering=False, debug=False,
                   num_devices=N_CORES)

    xt = nc.dram_tensor("xt", [D, T], DT_BF, kind="ExternalInput").ap()
    wkv = nc.dram_tensor("wkv", [D, 128], DT_BF, kind="ExternalInput").ap()
    wq = nc.dram_tensor("wq", [D, H], DT_BF, kind="ExternalInput").ap()
    masks = nc.dram_tensor("masks", [128, 256], DT_BF, kind="ExternalInput").ap()
    identb = nc.dram_tensor("identb", [128, 64], DT_BF, kind="ExternalInput").ap()
    y = nc.dram_tensor("y", [65, TL], DT_F32, kind="ExternalOutput").ap()

    with tile.TileContext(nc) as tc:
        _body(nc, tc, xt, wkv, wq, masks, identb, y)

    nc.compile()
    return nc


def _body(nc, tc, xt, wkv, wq, masks, identb, y):
    from contextlib import ExitStack

    es = ExitStack()
    with es:
        pp = es.enter_context(tc.tile_pool(name="persist", bufs=1))
        xt_sb = pp.tile([128, DCH * T], DT_BF)
        wkv_sb = pp.tile([128, DCH * 128], DT_BF)
        wq_sb = pp.tile([128, DCH * H], DT_BF)
        masks_sb = pp.tile([128, 256], DT_BF)
        identb_sb = pp.tile([128, 64], DT_BF)
        kvT_sb = pp.tile([128, T], DT_BF)       # rows 0:64 = K^T, 64:128 = V^T
        qT_sb = pp.tile([64, TL], DT_BF)
        vones_sb = pp.tile([128, NKT * 65], DT_BF)  # V tiles + ones col

        # ---- input DMAs ----
        # small constants first on the (idle-early) scalar SWDGE queue
        nc.scalar.dma_start(wkv_sb.rearrange("p (d t) -> p d t", t=128),
                            wkv.rearrange("(d p) t -> p d t", p=128))
        nc.scalar.dma_start(wq_sb.rearrange("p (d t) -> p d t", t=H),
                            wq.rearrange("(d p) t -> p d t", p=128))
        nc.scalar.dma_start(identb_sb[:], identb[:])
        nc.scalar.dma_start(masks_sb[:], masks[:])

        # x^T in 8 criticality-ordered 512-col slices on the sync queue
        xt_src = xt.rearrange("(d p) t -> p d t", p=128)
        xt_dst = xt_sb.rearrange("p (d t) -> p d t", t=T)
        for s in range(8):
            nc.sync.dma_start(xt_dst[:, :, s * 512:(s + 1) * 512],
                              xt_src[:, :, s * 512:(s + 1) * 512])

        nc.gpsimd.memset(vones_sb[:], 1.0)

        psum_kv = es.enter_context(
            tc.tile_pool(name="psum_kv", bufs=1, space="PSUM"))
        psum_vt = es.enter_context(
            tc.tile_pool(name="psum_vt", bufs=1, space="PSUM"))

        # even-block view of x^T for Q projection:
        # [p][d][u][wi] with wi = 256 (A block 128 | B block 128)
        xq_view = xt_sb.rearrange("p (d u wi) -> p d u wi", d=DCH, wi=256)

        def emit_q(u0, n):
            """Project q tiles u0..u0+n-1 (n*128 cols) from even blocks."""
            with tc.tile_pool(name=f"psum_q{u0}", bufs=1, space="PSUM") as pq_pool:
                pq = pq_pool.tile([64, n * 128], DT_F32, name=f"pq{u0}", tag="pq")
                for d in range(DCH):
                    nc.tensor.matmul(
                        pq[:],
                        lhsT=wq_sb[:, d * H:(d + 1) * H],
                        rhs=xq_view[:, d, u0:u0 + n, 0:128],
                        start=(d == 0), stop=(d == DCH - 1))
                nc.vector.tensor_copy(qT_sb[:, u0 * 128:(u0 + n) * 128], pq[:])

        def emit_kv(s):
            """K^T/V^T for permuted key tiles 4s..4s+3 (xt cols 512s..+512)."""
            pkv = psum_kv.tile([128, 512], DT_F32, name=f"pkv{s}", tag="pkv")
            for d in range(DCH):
                nc.tensor.matmul(
                    pkv[:],
                    lhsT=wkv_sb[:, d * 128:(d + 1) * 128],
                    rhs=xt_sb[:, d * T + s * 512: d * T + s * 512 + 512],
                    start=(d == 0), stop=(d == DCH - 1))
            nc.vector.tensor_copy(kvT_sb[:, s * 512:(s + 1) * 512], pkv[:])
            for t in range(4 * s, 4 * s + 4):
                pv = psum_vt.tile([128, 64], DT_BF, name=f"pv{t}", tag="pv")
                nc.tensor.transpose(pv[:],
                                    kvT_sb[64:128, t * 128:(t + 1) * 128],
                                    identb_sb[64:128, :])
                nc.vector.tensor_copy(vones_sb[:, t * 65: t * 65 + 64], pv[:])

        kv_state = [0]

        def ensure_kv(tile_idx):
            while kv_state[0] * 4 <= tile_idx:
                emit_kv(kv_state[0])
                kv_state[0] += 1

        def attention_phase(a, b):
            W = b - a
            u_hi = (b - 1) // 128
            with tc.tile_pool(name=f"ctx{a}", bufs=1, space="PSUM") as pc, \
                 tc.tile_pool(name=f"strip{a}", bufs=2, space="PSUM") as pstrip, \
                 tc.tile_pool(name=f"pT{a}", bufs=4) as ppT:
                ctx_ps = pc.tile([65, W], DT_F32, name=f"ctx{a}", tag="ctx")
                for u in range(u_hi + 1):
                    t0, t1 = 2 * u, 2 * u + 1
                    ensure_kv(t1)
                    c_lo = max(128 * u, a)
                    rel = c_lo - a
                    ps = pstrip.tile([128, 2 * W], DT_F32,
                                     name=f"ps{a}_{u}", tag="ps")
                    nc.tensor.matmul(ps[:, rel:W],
                                     lhsT=kvT_sb[0:64, t0 * 128:t0 * 128 + 128],
                                     rhs=qT_sb[:, c_lo:b],
                                     start=True, stop=True)
                    nc.tensor.matmul(ps[:, W + rel:2 * W],
                                     lhsT=kvT_sb[0:64, t1 * 128:t1 * 128 + 128],
                                     rhs=qT_sb[:, c_lo:b],
                                     start=True, stop=True)
                    pt = ppT.tile([128, 2 * W], DT_BF, name=f"pt{a}_{u}", tag="pt")
                    ps2 = ps.rearrange("p (h w) -> p h w", h=2)
                    pt2 = pt.rearrange("p (h w) -> p h w", h=2)
                    nc.scalar.activation(pt2[:, :, rel:W], ps2[:, :, rel:W],
                                         EXP, bias=0.0, scale=0.125)
                    if 128 * u >= a:  # diagonal pair lives in this phase
                        d0 = 128 * u - a
                        nc.vector.tensor_tensor(
                            pt[:, d0:d0 + 128], pt[:, d0:d0 + 128],
                            masks_sb[:, 0:128], MUL)
                        nc.vector.tensor_tensor(
                            pt[:, W + d0:W + d0 + 128], pt[:, W + d0:W + d0 + 128],
                            masks_sb[:, 128:256], MUL)
                    nc.tensor.matmul(ctx_ps[:, rel:W],
                                     lhsT=vones_sb[:, t0 * 65: t0 * 65 + 65],
                                     rhs=pt[:, rel:W],
                                     start=(u == 0), stop=False)
                    nc.tensor.matmul(ctx_ps[:, rel:W],
                                     lhsT=vones_sb[:, t1 * 65: t1 * 65 + 65],
                                     rhs=pt[:, W + rel:2 * W],
                                     start=False, stop=(u == u_hi))

                # epilogue: ship raw [num;den]^T for this phase, divide on host
                with tc.tile_pool(name=f"ep{a}", bufs=1) as pes:
                    cs = pes.tile([65, W], DT_F32, name=f"cs{a}", tag="cs")
                    nc.vector.tensor_copy(cs[:], ctx_ps[:])
                    nc.sync.dma_start(y[:, a:b], cs[:])

        # ---- phase structure (paced by xt slice arrival) ----
        emit_q(*QEMITS[0])          # xt slice 0
        attention_phase(*PHASES[0])  # kv slice 0
        attention_phase(*PHASES[1])
        emit_q(*QEMITS[1])          # xt slice 1
        attention_phase(*PHASES[2])  # kv slice 1
        emit_q(*QEMITS[2])          # xt slices 2,3
        attention_phase(*PHASES[3])  # kv slices 2,3
        emit_q(*QEMITS[3])          # xt slices 4,5
        attention_phase(*PHASES[4])  # kv slices 4,5
        emit_q(*QEMITS[4])          # xt slices 6,7
        attention_phase(*PHASES[5])  # kv slices 6,7


def _perm(r):
    """Permuted block order: pos 2u -> global 2u+r, pos 2u+1 -> 2u+1-r."""
    p = np.arange(NKT)
    if r == 1:
        p = p.reshape(-1, 2)[:, ::-1].reshape(-1)
    return p


_ROW_IDX = [np.concatenate([256 * u + 128 * r + np.arange(128)
                            for u in range(NU)]) for r in range(2)]


def _host_prep(inputs):
    x = np.asarray(inputs["x"], dtype=F32)
    Wk = np.asarray(inputs["Wk"], dtype=F32)
    Wq = np.asarray(inputs["Wq"], dtype=F32)
    Wv = np.asarray(inputs["Wv"], dtype=F32)

    wkv = np.ascontiguousarray(np.concatenate([Wk, Wv], axis=1)).astype(BF16)
    wq = np.ascontiguousarray(Wq).astype(BF16)
    identb = np.zeros((128, 64), dtype=F32)
    identb[64:128, :] = np.eye(64, dtype=F32)
    identb = identb.astype(BF16)

    ii = np.arange(128)[None, :]   # free dim: q index within tile
    cc = np.arange(128)[:, None]   # partition dim: key index within tile
    maskA = (cc <= ii)
    in_maps = []
    for c in range(N_CORES):
        b, r = c // 2, c % 2
        perm = _perm(r)
        xt_np = np.ascontiguousarray(
            x[b].T.reshape(D, NKT, 128)[:, perm, :].reshape(D, T)).astype(BF16)
        maskB = np.full((128, 128), bool(r))
        masks_np = np.concatenate([maskA, maskB], axis=1).astype(BF16)
        in_maps.append(dict(xt=xt_np, wkv=wkv, wq=wq,
                            masks=masks_np, identb=identb))
    return in_maps


def _gather(results):
    out = np.zeros((B, T, H), dtype=F32)
    for c in range(N_CORES):
        b, r = c // 2, c % 2
        yc = results[c]["y"]  # [65, TL]: rows 0:64 = ctx^T, row 64 = denom
        out[b, _ROW_IDX[r]] = (yc[:64, :] / yc[64:65, :]).T
    return out


_NC_CACHE = []


def _execute(inputs, trace=False):
    if not _NC_CACHE:
        _NC_CACHE.append(_build())
    nc = _NC_CACHE[0]
    in_maps = _host_prep(inputs)
    res = run_bass_kernel_spmd(nc, in_maps, core_ids=list(range(N_CORES)),
                               trace=trace)
    return _gather(res.results), res


def kernel(**inputs):
    out, _ = _execute(inputs, trace=False)
    return out


# revision 15
# speedup vs baseline: 1.0462x; 1.0462x over previous
"""Causal attention head (B=4, T=4096, D=1024, H=64) on 8 TRN2 NeuronCores.

Sharding: 2 cores per batch element, block-interleaved. Core role r in {0,1}
owns 128-row query blocks {[256u+128r, 256u+128r+128) : u in [0,16)}.

Host permutes x^T's 128-column blocks so each core sees its OWN query blocks
at even positions: permuted block 2u = global block 2u+r (own, "A"), block
2u+1 = global block 2u+1-r (partner, "B"). This makes the program
SPMD-uniform: q-tile u attends permuted key tiles {A(0..u), B(0..u)} where
A(u) carries a lower-triangular causal mask and B(u) an all-zero (r=0) /
all-one (r=1) data mask. Q projects straight out of the permuted x^T's even
blocks, so only ONE 8 MiB x^T load per core (no separate x_q load).

Per-core device program:
  - weights (wkv/wq/masks/ident) DMA'd first on the scalar queue;
    x^T [D,T] bf16 in 8 slices on the sync queue
  - K^T/V^T projection per 512-col slice (lhsT=[Wk|Wv]), lazily interleaved
    into the attention phases; V^T -> V via PE transposes into [128,65]
    tiles with a fused ones column (softmax denominator)
  - Q^T projection from even blocks via 2-level strided APs, paced by slice
    arrival
  - flash-style attention in 6 query-column phases (128/128/256/512/512/512
    wide). Per q-tile u, the A/B key-tile strips share one PSUM tile and a
    single exp (ScalarE, scale=1/8); causal masks multiply on the diagonal
    128 cols; ctx^T accumulated [65, W] in PSUM per phase
  - per-phase epilogue: copy ctx^T to SBUF, DMA out raw [num;den] rows
Host side: permute/cast inputs, gather + divide by denominator row.
"""

import numpy as np
import ml_dtypes

import concourse.tile as tile
import concourse.mybir as mybir
from concourse import bacc
from concourse.bass_utils import run_bass_kernel_spmd

BF16 = ml_dtypes.bfloat16
F32 = np.float32

B, T, D, H = 4, 4096, 1024, 64
TL = 2048          # local query columns per core
N_CORES = 8
NKT = T // 128     # 32 key tiles (permuted order)
NU = TL // 128     # 16 own query tiles
DCH = D // 128     # 8 contraction chunks
DT_BF = mybir.dt.bfloat16
DT_F32 = mybir.dt.float32
EXP = mybir.ActivationFunctionType.Exp
MUL = mybir.AluOpType.mult

# attention phases: absolute local q-col ranges, each width <= 512
PHASES = [(0, 128), (128, 256), (256, 512),
          (512, 1024), (1024, 1536), (1536, 2048)]
# Q emissions: (u0, ntiles) -> requires xt slices floor(u0/2) .. floor((u0+n-1)/2)
QEMITS = [(0, 2), (2, 2), (4, 4), (8, 4), (12, 4)]


def _build():
    nc = bacc.Bacc("TRN2", target_bir_lowering=False, debug=False,
                   num_devices=N_CORES)

    xt = nc.dram_tensor("xt", [D, T], DT_BF, kind="ExternalInput").ap()
    # w = [Wq | Wk | Wv] packed; mi = [maskA | maskB | identb] packed
    w = nc.dram_tensor("w", [D, 192], DT_BF, kind="ExternalInput").ap()
    mi = nc.dram_tensor("mi", [128, 320], DT_BF, kind="ExternalInput").ap()
    y = nc.dram_tensor("y", [65, TL], DT_F32, kind="ExternalOutput").ap()

    with tile.TileContext(nc) as tc:
        _body(nc, tc, xt, w, mi, y)

    nc.compile()
    return nc


def _body(nc, tc, xt, w, mi, y):
    from contextlib import ExitStack

    es = ExitStack()
    with es:
        pp = es.enter_context(tc.tile_pool(name="persist", bufs=1))
        xt_sb = pp.tile([128, DCH * T], DT_BF)
        w_sb = pp.tile([128, DCH * 192], DT_BF)   # per chunk: [Wq|Wk|Wv]
        mi_sb = pp.tile([128, 320], DT_BF)        # [maskA|maskB|identb]
        kvT_sb = pp.tile([128, T], DT_BF)       # rows 0:64 = K^T, 64:128 = V^T
        qT_sb = pp.tile([64, TL], DT_BF)
        vones_sb = pp.tile([128, NKT * 65], DT_BF)  # V tiles + ones col

        def wq_ap(d):
            return w_sb[:, d * 192: d * 192 + 64]

        def wkv_ap(d):
            return w_sb[:, d * 192 + 64: d * 192 + 192]

        # ---- input DMAs: everything on the sync queue, weights FIRST ----
        nc.sync.dma_start(w_sb.rearrange("p (d t) -> p d t", t=192),
                          w.rearrange("(d p) t -> p d t", p=128))
        nc.sync.dma_start(mi_sb[:], mi[:])

        # x^T in 8 sequential 512-col slices
        xt_src = xt.rearrange("(d p) t -> p d t", p=128)
        xt_dst = xt_sb.rearrange("p (d t) -> p d t", t=T)
        for s in range(8):
            nc.sync.dma_start(xt_dst[:, :, s * 512:(s + 1) * 512],
                              xt_src[:, :, s * 512:(s + 1) * 512])

        nc.gpsimd.memset(vones_sb[:], 1.0)

        # kv projection + V transposes share one PSUM bank pool
        psum_kv = es.enter_context(
            tc.tile_pool(name="psum_kv", bufs=1, space="PSUM"))

        # even-block view of x^T for Q projection:
        # [p][d][u][wi] with wi = 256 (A block 128 | B block 128)
        xq_view = xt_sb.rearrange("p (d u wi) -> p d u wi", d=DCH, wi=256)

        def emit_q(u0, n, pool=None):
            """Project q tiles u0..u0+n-1 (n*128 cols) from even blocks."""
            from contextlib import nullcontext
            cm = (nullcontext(pool) if pool is not None else
                  tc.tile_pool(name=f"psum_q{u0}", bufs=1, space="PSUM"))
            with cm as pq_pool:
                tag = "ps" if pool is not None else "pq"
                pq = pq_pool.tile([64, n * 128], DT_F32, name=f"pq{u0}", tag=tag)
                for d in range(DCH):
                    nc.tensor.matmul(
                        pq[:],
                        lhsT=wq_ap(d),
                        rhs=xq_view[:, d, u0:u0 + n, 0:128],
                        start=(d == 0), stop=(d == DCH - 1))
                nc.vector.tensor_copy(qT_sb[:, u0 * 128:(u0 + n) * 128], pq[:])

        def emit_kv(s):
            """K^T/V^T for permuted key tiles 4s..4s+3 (xt cols 512s..+512)."""
            pkv = psum_kv.tile([128, 512], DT_F32, name=f"pkv{s}", tag="pkv")
            for d in range(DCH):
                nc.tensor.matmul(
                    pkv[:],
                    lhsT=wkv_ap(d),
                    rhs=xt_sb[:, d * T + s * 512: d * T + s * 512 + 512],
                    start=(d == 0), stop=(d == DCH - 1))
            nc.vector.tensor_copy(kvT_sb[:, s * 512:(s + 1) * 512], pkv[:])
            for t in range(4 * s, 4 * s + 4):
                pv = psum_kv.tile([128, 64], DT_BF, name=f"pv{t}", tag="pkv")
                nc.tensor.transpose(pv[:],
                                    kvT_sb[64:128, t * 128:(t + 1) * 128],
                                    mi_sb[64:128, 256:320])
                nc.vector.tensor_copy(vones_sb[:, t * 65: t * 65 + 64], pv[:])

        kv_state = [0]

        def ensure_kv(tile_idx):
            while kv_state[0] * 4 <= tile_idx:
                emit_kv(kv_state[0])
                kv_state[0] += 1

        def attention_phase(a, b, interleave=None):
            W = b - a
            u_hi = (b - 1) // 128
            interleave = interleave or {}
            with tc.tile_pool(name=f"ctx{a}", bufs=1, space="PSUM") as pc, \
                 tc.tile_pool(name=f"strip{a}", bufs=3, space="PSUM") as pstrip, \
                 tc.tile_pool(name=f"pT{a}", bufs=6) as ppT:
                ctx_ps = pc.tile([65, W], DT_F32, name=f"ctx{a}", tag="ctx")
                for u in range(u_hi + 1):
                    if u in interleave:
                        interleave[u](pstrip)
                    t0, t1 = 2 * u, 2 * u + 1
                    ensure_kv(t1)
                    c_lo = max(128 * u, a)
                    rel = c_lo - a
                    ps = pstrip.tile([128, 2 * W], DT_F32,
                                     name=f"ps{a}_{u}", tag="ps")
                    nc.tensor.matmul(ps[:, rel:W],
                                     lhsT=kvT_sb[0:64, t0 * 128:t0 * 128 + 128],
                                     rhs=qT_sb[:, c_lo:b],
                                     start=True, stop=True)
                    nc.tensor.matmul(ps[:, W + rel:2 * W],
                                     lhsT=kvT_sb[0:64, t1 * 128:t1 * 128 + 128],
                                     rhs=qT_sb[:, c_lo:b],
                                     start=True, stop=True)
                    pt = ppT.tile([128, 2 * W], DT_BF, name=f"pt{a}_{u}", tag="pt")
                    ps2 = ps.rearrange("p (h w) -> p h w", h=2)
                    pt2 = pt.rearrange("p (h w) -> p h w", h=2)
                    nc.scalar.activation(pt2[:, :, rel:W], ps2[:, :, rel:W],
                                         EXP, bias=0.0, scale=0.125)
                    if 128 * u >= a:  # diagonal pair lives in this phase
                        d0 = 128 * u - a
                        nc.vector.tensor_tensor(
                            pt[:, d0:d0 + 128], pt[:, d0:d0 + 128],
                            mi_sb[:, 0:128], MUL)
                        nc.vector.tensor_tensor(
                            pt[:, W + d0:W + d0 + 128], pt[:, W + d0:W + d0 + 128],
                            mi_sb[:, 128:256], MUL)
                    nc.tensor.matmul(ctx_ps[:, rel:W],
                                     lhsT=vones_sb[:, t0 * 65: t0 * 65 + 65],
                                     rhs=pt[:, rel:W],
                                     start=(u == 0), stop=False)
                    nc.tensor.matmul(ctx_ps[:, rel:W],
                                     lhsT=vones_sb[:, t1 * 65: t1 * 65 + 65],
                                     rhs=pt[:, W + rel:2 * W],
                                     start=False, stop=(u == u_hi))

                # epilogue: ship raw [num;den]^T for this phase, divide on host
                with tc.tile_pool(name=f"ep{a}", bufs=1) as pes:
                    cs = pes.tile([65, W], DT_F32, name=f"cs{a}", tag="cs")
                    nc.vector.tensor_copy(cs[:], ctx_ps[:])
                    nc.sync.dma_start(y[:, a:b], cs[:])

        # ---- phase structure (paced by xt slice arrival) ----
        emit_q(0, 2)                 # xt slice 0
        attention_phase(*PHASES[0])  # kv slice 0
        attention_phase(*PHASES[1])
        emit_q(2, 2)                 # xt slice 1
        attention_phase(*PHASES[2])  # kv slice 1
        emit_q(4, 2)                 # xt slice 2
        emit_q(6, 2)                 # xt slice 3
        # ph3 emits kv slices 2,3 lazily; Q 8..11 interleaved mid-phase
        attention_phase(*PHASES[3],
                        interleave={5: lambda p: emit_q(8, 2, p),    # xt slice 4
                                    7: lambda p: emit_q(10, 2, p)})  # xt slice 5
        # ph4a emits kv slices 4,5 lazily; Q 12..15 interleaved mid-phase
        attention_phase(*PHASES[4],
                        interleave={8: lambda p: emit_q(12, 2, p),   # xt slice 6
                                    11: lambda p: emit_q(14, 2, p)})  # xt slice 7
        attention_phase(*PHASES[5])  # kv slices 6,7


def _perm(r):
    """Permuted block order: pos 2u -> global 2u+r, pos 2u+1 -> 2u+1-r."""
    p = np.arange(NKT)
    if r == 1:
        p = p.reshape(-1, 2)[:, ::-1].reshape(-1)
    return p


_ROW_IDX = [np.concatenate([256 * u + 128 * r + np.arange(128)
                            for u in range(NU)]) for r in range(2)]


def _host_prep(inputs):
    x = np.asarray(inputs["x"], dtype=F32)
    Wk = np.asarray(inputs["Wk"], dtype=F32)
    Wq = np.asarray(inputs["Wq"], dtype=F32)
    Wv = np.asarray(inputs["Wv"], dtype=F32)

    # w = [Wq | Wk | Wv]  [D, 192]
    w = np.ascontiguousarray(np.concatenate([Wq, Wk, Wv], axis=1)).astype(BF16)
    identb = np.zeros((128, 64), dtype=F32)
    identb[64:128, :] = np.eye(64, dtype=F32)

    ii = np.arange(128)[None, :]   # free dim: q index within tile
    cc = np.arange(128)[:, None]   # partition dim: key index within tile
    maskA = (cc <= ii).astype(F32)
    in_maps = []
    for c in range(N_CORES):
        b, r = c // 2, c % 2
        perm = _perm(r)
        xt_np = np.ascontiguousarray(
            x[b].T.reshape(D, NKT, 128)[:, perm, :].reshape(D, T)).astype(BF16)
        maskB = np.full((128, 128), float(r), dtype=F32)
        mi_np = np.concatenate([maskA, maskB, identb], axis=1).astype(BF16)
        in_maps.append(dict(xt=xt_np, w=w, mi=mi_np))
    return in_maps


def _gather(results):
    out = np.zeros((B, T, H), dtype=F32)
    for c in range(N_CORES):
        b, r = c // 2, c % 2
        yc = results[c]["y"]  # [65, TL]: rows 0:64 = ctx^T, row 64 = denom
        out[b, _ROW_IDX[r]] = (yc[:64, :] / yc[64:65, :]).T
    return out


_NC_CACHE = []


def _execute(inputs, trace=False):
    if not _NC_CACHE:
        _NC_CACHE.append(_build())
    nc = _NC_CACHE[0]
    in_maps = _host_prep(inputs)
    res = run_bass_kernel_spmd(nc, in_maps, core_ids=list(range(N_CORES)),
                               trace=trace)
    return _gather(res.results), res


def kernel(**inputs):
    out, _ = _execute(inputs, trace=False)
    return out


# revision 16
# speedup vs baseline: 1.0566x; 1.0099x over previous
"""Causal attention head (B=4, T=4096, D=1024, H=64) on 8 TRN2 NeuronCores.

Sharding: 2 cores per batch element, block-interleaved. Core role r in {0,1}
owns 128-row query blocks {[256u+128r, 256u+128r+128) : u in [0,16)}.

Host reorders x^T's 128-column blocks into [A-half | B-half]: the first 2048
columns are the core's OWN query blocks A(u) = global block 2u+r in order,
the last 2048 are the partner's B(u) = global block 2u+1-r. This makes the
program SPMD-uniform AND all access patterns contiguous: q-tile u attends
key tiles {A(0..u), B(0..u)} where A(u) carries a lower-triangular causal
mask and B(u) an all-zero (r=0) / all-one (r=1) data mask. Q projects from
the contiguous A-half, so only ONE 8 MiB x^T load per core.

Per-core device program:
  - weights [Wq|Wk|Wv] + masks/ident DMA'd first on the sync queue, then
    x^T in 8 512-col slices ordered A0,B0,A1,B1,A2,B2,A3,B3
  - K^T/V^T projection per 512-col slice (lhsT=[Wk|Wv]), lazily interleaved
    into the attention phases; V^T -> V via PE transposes into [128,65]
    tiles with a fused ones column (softmax denominator)
  - Q^T projection [64,512] per A-slice, paced by slice arrival
  - flash-style attention in phases over query columns; strips per key tile
    with 512-col chunks sharing the stationary operand; exp on ScalarE
    (scale=1/8) -> bf16; causal mask multiply on diagonal 128 cols; ctx^T
    accumulated [65, W] in PSUM with per-bank-group stops and epilogues.
    The tiny (128,256) phase runs LAST so the drain tail is short.
Host side: reorder/cast inputs, gather + divide by denominator row.
"""

import numpy as np
import ml_dtypes

import concourse.tile as tile
import concourse.mybir as mybir
from concourse import bacc
from concourse.bass_utils import run_bass_kernel_spmd

BF16 = ml_dtypes.bfloat16
F32 = np.float32

B, T, D, H = 4, 4096, 1024, 64
TL = 2048          # local query columns per core
N_CORES = 8
NKT = T // 128     # 32 key tiles
NU = TL // 128     # 16 own query tiles
DCH = D // 128     # 8 contraction chunks
DT_BF = mybir.dt.bfloat16
DT_F32 = mybir.dt.float32
EXP = mybir.ActivationFunctionType.Exp
MUL = mybir.AluOpType.mult


def _chunks512(a0, a1):
    """Split [a0, a1) at absolute multiples of 512 (PSUM bank boundaries)."""
    out = []
    while a0 < a1:
        a2 = min(a1, (a0 // 512 + 1) * 512)
        out.append((a0, a2))
        a0 = a2
    return out


def _build():
    nc = bacc.Bacc("TRN2", target_bir_lowering=False, debug=False,
                   num_devices=N_CORES)

    xt = nc.dram_tensor("xt", [D, T], DT_BF, kind="ExternalInput").ap()
    # w = [Wq | Wk | Wv]; mi = [maskA | maskB | identb]
    w = nc.dram_tensor("w", [D, 192], DT_BF, kind="ExternalInput").ap()
    mi = nc.dram_tensor("mi", [128, 320], DT_BF, kind="ExternalInput").ap()
    y = nc.dram_tensor("y", [65, TL], DT_F32, kind="ExternalOutput").ap()

    with tile.TileContext(nc) as tc:
        _body(nc, tc, xt, w, mi, y)

    nc.compile()
    return nc


def _body(nc, tc, xt, w, mi, y):
    from contextlib import ExitStack

    es = ExitStack()
    with es:
        pp = es.enter_context(tc.tile_pool(name="persist", bufs=1))
        xt_sb = pp.tile([128, DCH * T], DT_BF)
        w_sb = pp.tile([128, DCH * 192], DT_BF)   # per chunk: [Wq|Wk|Wv]
        mi_sb = pp.tile([128, 320], DT_BF)        # [maskA|maskB|identb]
        kvT_sb = pp.tile([128, T], DT_BF)   # cols: A tiles 0:2048, B 2048:4096
        qT_sb = pp.tile([64, TL], DT_BF)
        vones_sb = pp.tile([128, NKT * 65], DT_BF)  # slots: A u, B 16+u

        # ---- input DMAs: all on the sync queue, weights FIRST ----
        nc.sync.dma_start(w_sb.rearrange("p (d t) -> p d t", t=192),
                          w.rearrange("(d p) t -> p d t", p=128))
        nc.sync.dma_start(mi_sb[:], mi[:])

        # x^T slices: A0,B0,A1,B1,... (A-half cols [0,2048), B [2048,4096))
        xt_src = xt.rearrange("(d p) t -> p d t", p=128)
        xt_dst = xt_sb.rearrange("p (d t) -> p d t", t=T)
        for s in range(4):
            for half in (0, 2048):
                c0 = half + s * 512
                nc.sync.dma_start(xt_dst[:, :, c0:c0 + 512],
                                  xt_src[:, :, c0:c0 + 512])

        nc.gpsimd.memset(vones_sb[:], 1.0)

        psum_kv = es.enter_context(
            tc.tile_pool(name="psum_kv", bufs=1, space="PSUM"))
        psum_vt = es.enter_context(
            tc.tile_pool(name="psum_vt", bufs=1, space="PSUM"))

        def emit_q(s):
            """Q^T for q tiles 4s..4s+3 from A-half cols [512s, 512s+512)."""
            with tc.tile_pool(name=f"psum_q{s}", bufs=1, space="PSUM") as pqp:
                pq = pqp.tile([64, 512], DT_F32, name=f"pq{s}", tag="pq")
                for d in range(DCH):
                    nc.tensor.matmul(
                        pq[:],
                        lhsT=w_sb[:, d * 192: d * 192 + 64],
                        rhs=xt_sb[:, d * T + s * 512: d * T + s * 512 + 512],
                        start=(d == 0), stop=(d == DCH - 1))
                nc.vector.tensor_copy(qT_sb[:, s * 512:(s + 1) * 512], pq[:])

        def emit_kv(half, s):
            """K^T/V^T for tiles {half}(4s..4s+3); xt cols half*2048+512s.."""
            c0 = half * 2048 + s * 512
            k0 = half * 2048 + s * 512      # kvT_sb dst cols
            v0 = half * 16 + 4 * s          # vones slot base
            pkv = psum_kv.tile([128, 512], DT_F32, name=f"pkv{half}_{s}",
                               tag="pkv")
            for d in range(DCH):
                nc.tensor.matmul(
                    pkv[:],
                    lhsT=w_sb[:, d * 192 + 64: d * 192 + 192],
                    rhs=xt_sb[:, d * T + c0: d * T + c0 + 512],
                    start=(d == 0), stop=(d == DCH - 1))
            nc.vector.tensor_copy(kvT_sb[:, k0:k0 + 512], pkv[:])
            for i in range(4):
                pv = psum_vt.tile([128, 64], DT_BF, name=f"pv{v0 + i}", tag="pv")
                nc.tensor.transpose(
                    pv[:],
                    kvT_sb[64:128, k0 + i * 128: k0 + i * 128 + 128],
                    mi_sb[64:128, 256:320])
                nc.vector.tensor_copy(
                    vones_sb[:, (v0 + i) * 65: (v0 + i) * 65 + 64], pv[:])

        kv_state = [0]  # number of (A,B) slice pairs emitted

        def ensure_kv(u):
            while kv_state[0] * 4 <= u:
                emit_kv(0, kv_state[0])
                emit_kv(1, kv_state[0])
                kv_state[0] += 1

        def strip(pstrip, ppT, ctx_ps, a, b, u, half, first, last):
            """One key-tile strip: S^T chunks, exp, mask, ctx accumulate."""
            c_lo = max(128 * u, a)
            kcol = half * 2048 + u * 128    # K^T tile cols in kvT_sb
            vslot = half * 16 + u           # vones slot
            ps = pstrip.tile([128, b - a], DT_F32,
                             name=f"ps{a}_{half}_{u}", tag="ps")
            for (a0, a1) in _chunks512(c_lo, b):
                nc.tensor.matmul(ps[:, a0 - a: a1 - a],
                                 lhsT=kvT_sb[0:64, kcol:kcol + 128],
                                 rhs=qT_sb[:, a0:a1],
                                 start=True, stop=True)
            pt = ppT.tile([128, b - a], DT_BF, name=f"pt{a}_{half}_{u}",
                          tag="pt")
            nc.scalar.activation(pt[:, c_lo - a:], ps[:, c_lo - a:],
                                 EXP, bias=0.0, scale=0.125)
            if 128 * u >= a:  # diagonal tile lives in this phase
                d0 = 128 * u - a
                moff = half * 128
                nc.vector.tensor_tensor(
                    pt[:, d0:d0 + 128], pt[:, d0:d0 + 128],
                    mi_sb[:, moff:moff + 128], MUL)
            for (a0, a1) in _chunks512(c_lo, b):
                g = (a0 - a) // 512
                nc.tensor.matmul(
                    ctx_ps[:, a0 - a: a1 - a],
                    lhsT=vones_sb[:, vslot * 65: vslot * 65 + 65],
                    rhs=pt[:, a0 - a: a1 - a],
                    start=first,
                    stop=last[g] == (u, half))

        def attention_phase(a, b, after=None):
            """Phase over local q cols [a, b); after: {u: callback}."""
            W = b - a
            u_hi = (b - 1) // 128
            after = after or {}
            # last (u, half) contributing to ctx bank group g
            last = {}
            for g in range((W + 511) // 512):
                ug = min(u_hi, (a + 512 * g + 511) // 128)
                last[g] = (ug, 1)
            with tc.tile_pool(name=f"ctx{a}", bufs=1, space="PSUM") as pc, \
                 tc.tile_pool(name=f"strip{a}", bufs=2, space="PSUM") as pstrip, \
                 tc.tile_pool(name=f"pT{a}", bufs=4) as ppT, \
                 tc.tile_pool(name=f"ep{a}", bufs=2) as pes:
                ctx_ps = pc.tile([65, W], DT_F32, name=f"ctx{a}", tag="ctx")
                for u in range(u_hi + 1):
                    ensure_kv(u)
                    for half in (0, 1):
                        strip(pstrip, ppT, ctx_ps, a, b, u, half,
                              first=(u == 0 and half == 0), last=last)
                    # ship any ctx bank group that just stopped
                    for g, (ug, hg) in last.items():
                        if (u, 1) == (ug, hg):
                            g0, g1 = 512 * g, min(512 * (g + 1), W)
                            cs = pes.tile([65, g1 - g0], DT_F32,
                                          name=f"cs{a}_{g}", tag="cs")
                            nc.vector.tensor_copy(cs[:], ctx_ps[:, g0:g1])
                            nc.sync.dma_start(y[:, a + g0:a + g1], cs[:])
                    if u in after:
                        after[u]()

        # ---- phase structure (paced by xt slice arrival) ----
        emit_q(0)                         # A0 (1st DMA)
        attention_phase(0, 128)           # kv pair 0 lazily (A0, B0)
        attention_phase(256, 512,
                        after={1: lambda: emit_q(1)})   # A1 (3rd DMA)
        attention_phase(512, 1024,
                        after={5: lambda: emit_q(2),    # A2 (5th DMA)
                               7: lambda: emit_q(3)})   # A3 (7th DMA)
        attention_phase(1024, 2048)
        attention_phase(128, 256)         # tiny phase last: short drain tail


_ROW_IDX = [np.concatenate([256 * u + 128 * r + np.arange(128)
                            for u in range(NU)]) for r in range(2)]


def _host_prep(inputs):
    x = np.asarray(inputs["x"], dtype=F32)
    Wk = np.asarray(inputs["Wk"], dtype=F32)
    Wq = np.asarray(inputs["Wq"], dtype=F32)
    Wv = np.asarray(inputs["Wv"], dtype=F32)

    w = np.ascontiguousarray(np.concatenate([Wq, Wk, Wv], axis=1)).astype(BF16)
    identb = np.zeros((128, 64), dtype=F32)
    identb[64:128, :] = np.eye(64, dtype=F32)

    ii = np.arange(128)[None, :]   # free dim: q index within tile
    cc = np.arange(128)[:, None]   # partition dim: key index within tile
    maskA = (cc <= ii).astype(F32)
    in_maps = []
    for c in range(N_CORES):
        b, r = c // 2, c % 2
        xtb = x[b].T.reshape(D, NKT, 128)
        own = xtb[:, r::2, :].reshape(D, TL)        # A: global blocks 2u+r
        peer = xtb[:, 1 - r::2, :].reshape(D, TL)   # B: global blocks 2u+1-r
        xt_np = np.ascontiguousarray(
            np.concatenate([own, peer], axis=1)).astype(BF16)
        maskB = np.full((128, 128), float(r), dtype=F32)
        mi_np = np.concatenate([maskA, maskB, identb], axis=1).astype(BF16)
        in_maps.append(dict(xt=xt_np, w=w, mi=mi_np))
    return in_maps


def _gather(results):
    out = np.zeros((B, T, H), dtype=F32)
    for c in range(N_CORES):
        b, r = c // 2, c % 2
        yc = results[c]["y"]  # [65, TL]: rows 0:64 = ctx^T, row 64 = denom
        out[b, _ROW_IDX[r]] = (yc[:64, :] / yc[64:65, :]).T
    return out


_NC_CACHE = []


def _execute(inputs, trace=False):
    if not _NC_CACHE:
        _NC_CACHE.append(_build())
    nc = _NC_CACHE[0]
    in_maps = _host_prep(inputs)
    res = run_bass_kernel_spmd(nc, in_maps, core_ids=list(range(N_CORES)),
                               trace=trace)
    return _gather(res.results), res


def kernel(**inputs):
    out, _ = _execute(inputs, trace=False)
    return out


# revision 17
# speedup vs baseline: 1.4039x; 1.3287x over previous
"""Causal attention head (B=4, T=4096, D=1024, H=64) on 8 TRN2 NeuronCores.

Sharding: 2 cores per batch element. Within a batch, core role r in {0,1}
owns the interleaved query rows {256*v + 2*i + r : v in [0,16), i in [0,128)}.
This gives every core an IDENTICAL instruction stream (SPMD-uniform):
virtual query tile v always attends to exactly 2*v+2 key tiles of 128, with
a role-dependent (data, not code) causal mask on the last two key tiles.

Per-core device program:
  - load x^T (full batch, [D,T] bf16) and x_q^T (own rows, [D,2048] bf16)
  - K^T/V^T projection (full T) via one matmul pass with lhsT=[Wk|Wv]
  - Q^T projection (local 2048 cols)
  - V^T -> V via PE transposes; V tiles stored as [128,65] with a ones column
    (fused softmax denominator)
  - flash-style attention, two query-column half phases (PSUM budget), key
    tile outer within each: S^T strips [128k x Nq] in PSUM, exp on ScalarE
    (scale=1/8) -> P^T bf16, causal mask multiply on the diagonal 128 cols,
    context accumulated as ctx^T[65, 1024] in PSUM per half
  - epilogue per half: PE-transpose ctx^T back to [q,65], multiply by
    reciprocal of the ones-row sum, DMA out [2048, 64] f32
Host side: shard/cast/transpose inputs, gather + re-interleave outputs.
"""

import numpy as np
import ml_dtypes

import concourse.tile as tile
import concourse.mybir as mybir
from concourse import bacc
from concourse.bass_utils import run_bass_kernel_spmd

BF16 = ml_dtypes.bfloat16
F32 = np.float32

B, T, D, H = 4, 4096, 1024, 64
TL = 2048          # local query columns per core
N_CORES = 8
NKT = T // 128     # 32 key tiles
NV = TL // 128     # 16 virtual query tiles
DCH = D // 128     # 8 contraction chunks
DT_BF = mybir.dt.bfloat16
DT_F32 = mybir.dt.float32
EXP = mybir.ActivationFunctionType.Exp
MUL = mybir.AluOpType.mult


def _chunks512(a0, a1):
    """Split [a0, a1) at absolute multiples of 512 (PSUM bank boundaries)."""
    out = []
    while a0 < a1:
        a2 = min(a1, (a0 // 512 + 1) * 512)
        out.append((a0, a2))
        a0 = a2
    return out


def _build():
    nc = bacc.Bacc("TRN2", target_bir_lowering=False, debug=False,
                   num_devices=N_CORES)

    xt = nc.dram_tensor("xt", [D, T], DT_BF, kind="ExternalInput").ap()
    xtq = nc.dram_tensor("xtq", [D, TL], DT_BF, kind="ExternalInput").ap()
    wkv = nc.dram_tensor("wkv", [D, 128], DT_BF, kind="ExternalInput").ap()
    wq = nc.dram_tensor("wq", [D, H], DT_BF, kind="ExternalInput").ap()
    masks = nc.dram_tensor("masks", [128, 256], DT_BF, kind="ExternalInput").ap()
    identb = nc.dram_tensor("identb", [128, 64], DT_BF, kind="ExternalInput").ap()
    y = nc.dram_tensor("y", [65, TL], DT_F32, kind="ExternalOutput").ap()

    with tile.TileContext(nc) as tc:
        _body(nc, tc, xt, xtq, wkv, wq, masks, identb, y)

    nc.compile()
    return nc


def _body(nc, tc, xt, xtq, wkv, wq, masks, identb, y):
    from contextlib import ExitStack

    es = ExitStack()
    with es:
        pp = es.enter_context(tc.tile_pool(name="persist", bufs=1))
        xt_sb = pp.tile([128, DCH * T], DT_BF)
        xtq_sb = pp.tile([128, DCH * TL], DT_BF)
        wkv_sb = pp.tile([128, DCH * 128], DT_BF)
        wq_sb = pp.tile([128, DCH * H], DT_BF)
        masks_sb = pp.tile([128, 256], DT_BF)
        identb_sb = pp.tile([128, 64], DT_BF)
        kvT_sb = pp.tile([128, T], DT_BF)       # rows 0:64 = K^T, 64:128 = V^T
        qT_sb = pp.tile([64, TL], DT_BF)
        vones_sb = pp.tile([128, NKT * 65], DT_BF)  # V tiles + ones col

        # ---- input DMAs (program order == DMA issue order) ----
        # batched >=1MiB transfers: [d, p, c] <-> [p, d*stride + c] 3D APs
        xt_src = xt.rearrange("(d p) t -> p d t", p=128)
        xt_dst = xt_sb.rearrange("p (d t) -> p d t", t=T)
        xtq_src = xtq.rearrange("(d p) t -> p d t", p=128)
        xtq_dst = xtq_sb.rearrange("p (d t) -> p d t", t=TL)

        # constants on the (otherwise idle) GpSimd SWDGE queue so they
        # don't serialize ahead of the big x^T loads in the sync FIFO
        nc.gpsimd.dma_start(wq_sb.rearrange("p (d t) -> p d t", t=H),
                            wq.rearrange("(d p) t -> p d t", p=128))
        nc.gpsimd.dma_start(wkv_sb.rearrange("p (d t) -> p d t", t=128),
                            wkv.rearrange("(d p) t -> p d t", p=128))
        nc.gpsimd.dma_start(identb_sb[:], identb[:])
        nc.gpsimd.dma_start(masks_sb[:], masks[:])

        def dma_xtq_slice(s):
            nc.sync.dma_start(xtq_dst[:, :, s * 512:(s + 1) * 512],
                              xtq_src[:, :, s * 512:(s + 1) * 512])

        def dma_xt_slice(s):
            nc.sync.dma_start(xt_dst[:, :, s * 512:(s + 1) * 512],
                              xt_src[:, :, s * 512:(s + 1) * 512])

        # criticality order: q cols [0,1024) -> xt slices 0-3 -> rest
        for s in (0, 1):
            dma_xtq_slice(s)
        for s in (0, 1, 2, 3):
            dma_xt_slice(s)
        for s in (2, 3):
            dma_xtq_slice(s)
        for s in (4, 5, 6, 7):
            dma_xt_slice(s)

        nc.vector.memset(vones_sb[:], 1.0)

        psum_kv = es.enter_context(
            tc.tile_pool(name="psum_kv", bufs=1, space="PSUM"))
        psum_vt = es.enter_context(
            tc.tile_pool(name="psum_vt", bufs=1, space="PSUM"))

        def emit_q_slices(slices):
            with tc.tile_pool(name="psum_q", bufs=2, space="PSUM") as psum_q:
                for s in slices:
                    pq = psum_q.tile([64, 512], DT_F32, name=f"pq{s}", tag="pq")
                    for d in range(DCH):
                        nc.tensor.matmul(
                            pq[:],
                            lhsT=wq_sb[:, d * H:(d + 1) * H],
                            rhs=xtq_sb[:, d * TL + s * 512: d * TL + s * 512 + 512],
                            start=(d == 0), stop=(d == DCH - 1))
                    nc.vector.tensor_copy(qT_sb[:, s * 512:(s + 1) * 512], pq[:])

        def emit_kv_slice(s):
            pkv = psum_kv.tile([128, 512], DT_F32, name=f"pkv{s}", tag="pkv")
            for d in range(DCH):
                nc.tensor.matmul(
                    pkv[:],
                    lhsT=wkv_sb[:, d * 128:(d + 1) * 128],
                    rhs=xt_sb[:, d * T + s * 512: d * T + s * 512 + 512],
                    start=(d == 0), stop=(d == DCH - 1))
            nc.vector.tensor_copy(kvT_sb[:, s * 512:(s + 1) * 512], pkv[:])
            for t in range(4 * s, 4 * s + 4):
                pv = psum_vt.tile([128, 64], DT_BF, name=f"pv{t}", tag="pv")
                nc.tensor.transpose(pv[:],
                                    kvT_sb[64:128, t * 128:(t + 1) * 128],
                                    identb_sb[64:128, :])
                nc.vector.tensor_copy(vones_sb[:, t * 65: t * 65 + 64], pv[:])

        def attention_phase(h, kv_emit_at):
            """Strips (j, h) for all valid j; ctx^T half [65, 1024] in PSUM."""
            base = 1024 * h
            with tc.tile_pool(name=f"psum_ctx{h}", bufs=1, space="PSUM") as pc:
                ctx_ps = pc.tile([65, 1024], DT_F32, name=f"ctx{h}", tag="ctx")
                with tc.tile_pool(name=f"psum_strip{h}", bufs=2, space="PSUM") as pstrip, \
                     tc.tile_pool(name=f"pT{h}", bufs=4) as ppT:
                    _attention_strips(h, base, ctx_ps, pstrip, ppT, kv_emit_at)

                # epilogue for this half: ship raw [num;den]^T, divide on host
                with tc.tile_pool(name=f"ep_sb{h}", bufs=1) as pes:
                    cs = pes.tile([65, 1024], DT_F32, name=f"cs{h}", tag="cs")
                    nc.vector.tensor_copy(cs[:], ctx_ps[:])
                    nc.sync.dma_start(y[:, base:base + 1024], cs[:])

        def _attention_strips(h, base, ctx_ps, pstrip, ppT, kv_emit_at):
                for j in range(NKT):
                    if j in kv_emit_at:
                        emit_kv_slice(kv_emit_at[j])
                    q0 = 128 * (j // 2)
                    c_lo = max(q0, base)
                    c_hi = base + 1024
                    if c_lo >= c_hi:
                        continue
                    ps = pstrip.tile([128, 1024], DT_F32,
                                     name=f"ps{h}_{j}", tag="ps")
                    for (a0, a1) in _chunks512(c_lo, c_hi):
                        nc.tensor.matmul(
                            ps[:, a0 - base: a1 - base],
                            lhsT=kvT_sb[0:64, j * 128:(j + 1) * 128],
                            rhs=qT_sb[:, a0:a1],
                            start=True, stop=True)
                    pt = ppT.tile([128, 1024], DT_BF, name=f"pt{h}_{j}", tag="pt")
                    nc.scalar.activation(pt[:, c_lo - base: 1024],
                                         ps[:, c_lo - base: 1024],
                                         EXP, bias=0.0, scale=0.125)
                    if c_lo == q0:  # diagonal tile: causal mask, first 128 cols
                        moff = (j % 2) * 128
                        nc.vector.tensor_tensor(
                            pt[:, c_lo - base: c_lo - base + 128],
                            pt[:, c_lo - base: c_lo - base + 128],
                            masks_sb[:, moff: moff + 128],
                            MUL)
                    for (a0, a1) in _chunks512(c_lo, c_hi):
                        g = a0 // 512
                        nc.tensor.matmul(
                            ctx_ps[:, a0 - base: a1 - base],
                            lhsT=vones_sb[:, j * 65: j * 65 + 65],
                            rhs=pt[:, a0 - base: a1 - base],
                            start=(j == 0),
                            stop=(j == min(NKT - 1, 8 * g + 7)))

        # ---- phase structure ----
        emit_q_slices([0, 1])
        emit_kv_slice(0)
        emit_kv_slice(1)
        # phase 1: q cols [0, 1024); needs kv slices 0-3
        attention_phase(0, kv_emit_at={4: 2, 8: 3})
        emit_q_slices([2, 3])
        # phase 2: q cols [1024, 2048); needs kv slices 4-7
        attention_phase(1, kv_emit_at={12: 4, 16: 5, 20: 6, 24: 7})


_ROW_IDX = [np.array([256 * v + 2 * i + r for v in range(NV) for i in range(128)])
            for r in range(2)]


def _host_prep(inputs):
    x = np.asarray(inputs["x"], dtype=F32)
    Wk = np.asarray(inputs["Wk"], dtype=F32)
    Wq = np.asarray(inputs["Wq"], dtype=F32)
    Wv = np.asarray(inputs["Wv"], dtype=F32)

    wkv = np.ascontiguousarray(np.concatenate([Wk, Wv], axis=1)).astype(BF16)
    wq = np.ascontiguousarray(Wq).astype(BF16)
    identb = np.zeros((128, 64), dtype=F32)
    identb[64:128, :] = np.eye(64, dtype=F32)
    identb = identb.astype(BF16)

    ii = np.arange(128)[None, :]
    cc = np.arange(128)[:, None]
    in_maps = []
    for c in range(N_CORES):
        b, r = c // 2, c % 2
        xt_np = np.ascontiguousarray(x[b].T).astype(BF16)
        xtq_np = np.ascontiguousarray(x[b][_ROW_IDX[r]].T).astype(BF16)
        maskA = (cc <= 2 * ii + r)
        maskB = (cc + 128 <= 2 * ii + r)
        masks_np = np.concatenate([maskA, maskB], axis=1).astype(BF16)
        in_maps.append(dict(xt=xt_np, xtq=xtq_np, wkv=wkv, wq=wq,
                            masks=masks_np, identb=identb))
    return in_maps


def _gather(results):
    out = np.zeros((B, T, H), dtype=F32)
    for c in range(N_CORES):
        b, r = c // 2, c % 2
        yc = results[c]["y"]  # [65, TL]: rows 0:64 = ctx^T, row 64 = denom
        out[b, _ROW_IDX[r]] = (yc[:64, :] / yc[64:65, :]).T
    return out


_NC_CACHE = []


def _execute(inputs, trace=False):
    if not _NC_CACHE:
        _NC_CACHE.append(_build())
    nc = _NC_CACHE[0]
    in_maps = _host_prep(inputs)
    res = run_bass_kernel_spmd(nc, in_maps, core_ids=list(range(N_CORES)),
                               trace=trace)
    return _gather(res.results), res


def kernel(**inputs):
    out, _ = _execute(inputs, trace=False)
    return out
